# revision 1
# baseline (speedup 1.0000x reference)
"""Trainium2 Bass kernel for the non-local-attention block (nn_DNL_74234214744693).

Reference computation (B=4, C=64, H=W=64, N=H*W=4096):
    k = conv1x1(x,kw,kb); k_wh = k - mean_j(k)
    q = conv1x1(x,qw,qb); q_wh = q - mean_j(q)
    qk[b,i,j] = sum_c k_wh[b,c,i] q_wh[b,c,j]
    m  = conv1x1(x,mw,mb) -> [B,N];  mm[b,i,j] = m[b,i]*m[b,j]
    f  = softmax(qk, axis=-1) + softmax(mm, axis=0)   # second softmax over BATCH
    y  = einsum('bci,bij->bcj', v, f) + BN(conv1x1(x,ww,wb))

Key algebraic facts used:
  * softmax_j(k_whT q_wh) == softmax_j(k_whT q_raw): the q-mean term is constant
    along j's softmax rows, so only k needs whitening.
  * softmax_j normalizer Z1[i] indexes the contraction dim, so y1 = (v/Z1) @ e1.
  * batch softmax: f2[b] = e2_b * R with e2_b = exp(m_b_i m_b_j), R = 1/D.

Sharding: each of 8 cores owns a 512-row i-slice of the [N,N] maps for ALL 4
batch samples (exp work is perfectly balanced, no duplication, no collectives).
Each core emits a partial y [4,64,4096]; host sums the 8 partials.
The conv+BN residual is folded into the output matmul with weights pre-scaled
by 1/8 (so the host-side sum reconstructs it exactly once).
"""

import functools

import numpy as np
import ml_dtypes

N_CORES = 8
B, C, H, W = 4, 64, 64, 64
N = H * W                 # 4096
SL = N // N_CORES         # 512  rows of the attention map per core
NIT = SL // 128           # 4    128-row tiles per core
NJQ = 4                   # 1024-wide column blocks in phase B
JQ = N // NJQ             # 1024
EPS = 1e-5

BF16 = ml_dtypes.bfloat16


def _build_program():
    import concourse.bass as bass
    import concourse.tile as tile
    from concourse import bacc, mybir

    dt = mybir.dt
    AF = mybir.ActivationFunctionType
    ALU = mybir.AluOpType
    AX = mybir.AxisListType

    nc = bacc.Bacc("TRN2", target_bir_lowering=False, debug=False,
                   enable_asserts=False, num_devices=1)

    # ---------------- DRAM I/O ----------------
    x_ext = nc.dram_tensor("x_ext", [B, C + 1, N], dt.bfloat16, kind="ExternalInput")
    xsl_ext = nc.dram_tensor("xsl_ext", [B, C + 1, SL], dt.bfloat16, kind="ExternalInput")
    qmT = nc.dram_tensor("qmT", [C + 1, C + 1], dt.bfloat16, kind="ExternalInput")
    kT = nc.dram_tensor("kT", [C + 1, C], dt.bfloat16, kind="ExternalInput")
    vmT = nc.dram_tensor("vmT", [C + 1, C + 1], dt.bfloat16, kind="ExternalInput")
    wT = nc.dram_tensor("wT", [C + 1, C], dt.bfloat16, kind="ExternalInput")
    y_part = nc.dram_tensor("y_part", [B, C, N], dt.float32, kind="ExternalOutput")

    with tile.TileContext(nc) as tc:
        from contextlib import ExitStack

        with ExitStack() as top:
            # ---------- persistent pools ----------
            consts = top.enter_context(tc.tile_pool(name="consts", bufs=1))
            p_kwh = top.enter_context(tc.tile_pool(name="p_kwh", bufs=2))
            p_vT = top.enter_context(tc.tile_pool(name="p_vT", bufs=B * NIT))
            p_v1p = top.enter_context(tc.tile_pool(name="p_v1p", bufs=B * NIT))
            p_mcol = top.enter_context(tc.tile_pool(name="p_mcol", bufs=B * NIT))
            p_f1 = top.enter_context(tc.tile_pool(name="p_f1", bufs=B * NIT))
            p_small = top.enter_context(tc.tile_pool(name="p_small", bufs=B * 4))
            dram = top.enter_context(tc.tile_pool(name="dram", bufs=1, space="DRAM"))

            sb_qmT = consts.tile([C + 1, C + 1], dt.bfloat16)
            sb_kT = consts.tile([C + 1, C], dt.bfloat16)
            sb_vmT = consts.tile([C + 1, C + 1], dt.bfloat16)
            sb_wT = consts.tile([C + 1, C], dt.bfloat16)
            nc.scalar.dma_start(sb_qmT, qmT.ap())
            nc.scalar.dma_start(sb_kT, kT.ap())
            nc.scalar.dma_start(sb_vmT, vmT.ap())
            nc.scalar.dma_start(sb_wT, wT.ap())

            md = dram.tile([B, N], dt.bfloat16)  # m values, for broadcast DMA

            k_wh = [p_kwh.tile([C, SL], dt.bfloat16, name=f"k_wh{b}", tag="k_wh") for b in range(B)]
            v_T = [[p_vT.tile([128, C], dt.bfloat16, name=f"v_T{b}_{i}", tag="v_T") for i in range(NIT)] for b in range(B)]
            v1p = [[p_v1p.tile([128, C], dt.bfloat16, name=f"v1p{b}_{i}", tag="v1p") for i in range(NIT)] for b in range(B)]
            m_col = [[p_mcol.tile([128, 1], dt.float32, name=f"m_col{b}_{i}", tag="m_col") for i in range(NIT)] for b in range(B)]
            f1 = [[p_f1.tile([128, N], dt.bfloat16, name=f"f1_{b}_{i}", tag="f1") for i in range(NIT)] for b in range(B)]
            negku = [p_small.tile([C, 1], dt.float32, name=f"negku{b}", tag="negku") for b in range(B)]

            # ---------- phases 0+A interleaved per b: convs then qk/exp ----------
            with ExitStack() as ph0:
                p_q = ph0.enter_context(tc.tile_pool(name="p_q", bufs=2))
                p_x = ph0.enter_context(tc.tile_pool(name="p_x", bufs=2))
                p_xsl = ph0.enter_context(tc.tile_pool(name="p_xsl", bufs=2))
                psP = ph0.enter_context(tc.tile_pool(name="psP", bufs=2, space="PSUM"))
                p_t0 = ph0.enter_context(tc.tile_pool(name="p_t0", bufs=8))
                p_z = ph0.enter_context(tc.tile_pool(name="p_z", bufs=8))

                def dma_phase(b):
                    x_sb = p_x.tile([C + 1, N], dt.bfloat16, name=f"x_sb{b}", tag="x_sb")
                    xsl_sb = p_xsl.tile([C + 1, SL], dt.bfloat16, name=f"xsl_sb{b}", tag="xsl_sb")
                    nc.sync.dma_start(x_sb, x_ext.ap()[b])
                    nc.sync.dma_start(xsl_sb, xsl_ext.ap()[b])
                    return x_sb, xsl_sb

                def q_block(b, x_sb, q_store):
                    for half in range(2):
                        ps_q = psP.tile([128, 2048], dt.float32, name=f"ps_q{b}_{half}", tag="psP")
                        for k4 in range(4):
                            j0 = half * 2048 + k4 * 512
                            nc.tensor.matmul(ps_q[0:C + 1, k4 * 512:(k4 + 1) * 512],
                                             sb_qmT, x_sb[:, j0:j0 + 512],
                                             start=True, stop=True)
                        dst = q_store[:, half * 2048:(half + 1) * 2048]
                        nc.vector.tensor_copy(dst, ps_q[0:C + 1, :])

                def conv_phase(b, x_sb, xsl_sb):
                    q_store = p_q.tile([C + 1, N], dt.bfloat16, name=f"q_store{b}", tag="q_store")

                    # xu = mean_j(x) (row 64 = ones -> mean 1.0)
                    xu_f = p_t0.tile([C + 1, 1], dt.float32, tag="t0")
                    xu_bf = p_t0.tile([C + 1, 1], dt.bfloat16, tag="t0b")
                    nc.vector.tensor_reduce(xu_f, x_sb, axis=AX.X, op=ALU.add)
                    nc.vector.tensor_scalar_mul(xu_bf, xu_f, 1.0 / N)

                    # misc psum slot: ku + v/m convs + k conv packed into one tile
                    ps_m = psP.tile([128, 2048], dt.float32, name=f"ps_m{b}", tag="psP")
                    nc.tensor.matmul(ps_m[0:C, 1536:1537], sb_kT, xu_bf,
                                     start=True, stop=True)
                    nc.vector.tensor_scalar_mul(negku[b], ps_m[0:C, 1536:1537], -1.0)
                    for it in range(NIT):
                        fo = (it // 2) * 512 + (it % 2) * 256
                        nc.tensor.matmul(ps_m[:, fo:fo + C + 1],
                                         xsl_sb[:, it * 128:(it + 1) * 128],
                                         sb_vmT, start=True, stop=True)
                    nc.tensor.matmul(ps_m[0:C, 1024:1536], sb_kT, xsl_sb,
                                     start=True, stop=True)
                    for it in range(NIT):
                        fo = (it // 2) * 512 + (it % 2) * 256
                        nc.vector.tensor_copy(v_T[b][it], ps_m[:, fo:fo + C])
                        nc.vector.tensor_copy(m_col[b][it], ps_m[:, fo + C:fo + C + 1])
                    nc.vector.tensor_scalar(k_wh[b], ps_m[0:C, 1024:1536],
                                            scalar1=negku[b], scalar2=None, op0=ALU.add)

                    # q_raw (rows 0..63) and m_row (row 64)
                    q_block(b, x_sb, q_store)

                    # stash m (bf16) in DRAM for later broadcast DMA
                    nc.sync.dma_start(md[b], q_store[C:C + 1, :])
                    return q_store

                def qk_phase(b, q_store, its):
                    # qk -> e1 (bf16) + row sums -> v1p
                    for it in its:
                        zp = [p_z.tile([128, 1], dt.float32, name=f"zp{j}", tag="zp") for j in range(2)]
                        for jh in range(2):
                            ps_qk = psP.tile([128, 2048], dt.float32, name="ps_qk", tag="psP")
                            for k4 in range(4):
                                j0 = jh * 2048 + k4 * 512
                                nc.tensor.matmul(
                                    ps_qk[:, k4 * 512:(k4 + 1) * 512],
                                    k_wh[b][:, it * 128:(it + 1) * 128],
                                    q_store[0:C, j0:j0 + 512],
                                    start=True, stop=True)
                            nc.scalar.activation(
                                f1[b][it][:, jh * 2048:(jh + 1) * 2048],
                                ps_qk, AF.Exp, accum_out=zp[jh])
                        z1 = p_z.tile([128, 1], dt.float32)
                        rz = p_z.tile([128, 1], dt.float32)
                        nc.vector.tensor_tensor(z1, zp[0], zp[1], op=ALU.add)
                        nc.vector.reciprocal_approx_fast(rz, z1)
                        nc.vector.tensor_scalar_mul(v1p[b][it], v_T[b][it], rz)

                # per-b: convs then qk; conv(b+1) is emitted between
                # qk(b)'s it=0..2 and it=3 so its psum slots and DVE copies
                # complete under the last e1 exps; its DMAs are issued a
                # phase early.
                dmas_cur = dma_phase(0)
                q_cur = conv_phase(0, *dmas_cur)
                for b in range(B):
                    if b + 1 < B:
                        dmas_next = dma_phase(b + 1)
                        qk_phase(b, q_cur, range(NIT - 1))
                        q_next = conv_phase(b + 1, *dmas_next)
                        qk_phase(b, q_cur, [NIT - 1])
                        q_cur = q_next
                    else:
                        qk_phase(b, q_cur, range(NIT))

            # ---------- phase B: e2/D/R/f2 + output matmuls ----------
            with ExitStack() as phB:
                psY = phB.enter_context(tc.tile_pool(name="psY", bufs=8, space="PSUM"))
                p_mbc = phB.enter_context(tc.tile_pool(name="p_mbc", bufs=8))
                p_e2 = phB.enter_context(tc.tile_pool(name="p_e2", bufs=14))
                p_dr = phB.enter_context(tc.tile_pool(name="p_dr", bufs=1))
                p_rr = phB.enter_context(tc.tile_pool(name="p_rr", bufs=1))
                p_rb = phB.enter_context(tc.tile_pool(name="p_rb", bufs=2))
                p_xw = phB.enter_context(tc.tile_pool(name="p_xw", bufs=5))
                p_out = phB.enter_context(tc.tile_pool(name="p_out", bufs=2))

                for jq in range(NJQ):
                    jsl = slice(jq * JQ, (jq + 1) * JQ)
                    m_bc = []
                    for b in range(B):
                        t = p_mbc.tile([128, JQ], dt.bfloat16, name="m_bc", tag="m_bc")
                        nc.sync.dma_start(t, md[b:b + 1, jsl].to_broadcast([128, JQ]))
                        m_bc.append(t)
                    x_wx = []
                    for b in range(B):
                        t = p_xw.tile([C + 1, JQ], dt.bfloat16, name="x_wx", tag="x_wx")
                        nc.sync.dma_start(t, x_ext.ap()[b][:, jsl])
                        x_wx.append(t)

                    ps_y = [[psY.tile([C, 512], dt.float32, name=f"ps_y{b}_{h}", tag="ps_y")
                             for h in range(2)] for b in range(B)]
                    # wx residual first: it is f2-independent, opens each
                    # accumulation group early so the group closes right
                    # after the last f2 matmul (shorter per-jq tail).
                    for b in range(B):
                        for h in range(2):
                            cs = slice(h * 512, (h + 1) * 512)
                            nc.tensor.matmul(ps_y[b][h], sb_wT, x_wx[b][:, cs],
                                             start=True, stop=False)
                    for it in range(NIT):
                        # e2_b = exp(m_i * m_j); D = sum_b e2; R = 1/D;
                        # f2_b = e2_b * R (in place), consumed immediately below.
                        e2 = [p_e2.tile([128, JQ], dt.bfloat16, name=f"e2_{b}", tag="e2") for b in range(B)]
                        for b in range(B):
                            nc.scalar.activation(e2[b], m_bc[b], AF.Exp,
                                                 scale=m_col[b][it])
                        dsum = p_dr.tile([128, JQ], dt.bfloat16)
                        rr = p_rr.tile([128, JQ], dt.float32)
                        nc.vector.tensor_tensor(dsum, e2[0], e2[1], op=ALU.add)
                        nc.vector.tensor_tensor(dsum, dsum, e2[2], op=ALU.add)
                        nc.vector.tensor_tensor(rr, dsum, e2[3], op=ALU.add)
                        nc.vector.reciprocal_approx_fast(rr, rr)
                        rrb = p_rb.tile([128, JQ], dt.bfloat16)
                        nc.vector.tensor_copy(rrb, rr)
                        for b in range(B):
                            eng = nc.vector if b < 2 else nc.gpsimd
                            eng.tensor_tensor(e2[b], e2[b], rrb, op=ALU.mult)
                        for b in range(B):
                            for h in range(2):
                                cs = slice(h * 512, (h + 1) * 512)
                                js = slice(jq * JQ + h * 512, jq * JQ + (h + 1) * 512)
                                nc.tensor.matmul(ps_y[b][h], v1p[b][it],
                                                 f1[b][it][:, js],
                                                 start=False, stop=False)
                                nc.tensor.matmul(ps_y[b][h], v_T[b][it],
                                                 e2[b][:, cs],
                                                 start=False,
                                                 stop=(it == NIT - 1))

                    for b in range(B):
                        out_sb = p_out.tile([C, JQ], dt.float32)
                        for h in range(2):
                            cs = slice(h * 512, (h + 1) * 512)
                            nc.scalar.copy(out_sb[:, cs], ps_y[b][h])
                        nc.gpsimd.dma_start(y_part.ap()[b][:, jsl], out_sb)

    nc.compile()
    return nc


@functools.lru_cache(maxsize=1)
def _get_program():
    return _build_program()


def _prep_inputs(inputs):
    x = np.asarray(inputs["x"], np.float32).reshape(B, C, N)
    ones = np.ones((B, 1, N), np.float32)
    x_ext = np.concatenate([x, ones], axis=1).astype(BF16)          # [B,65,N]

    qw = np.asarray(inputs["qw"], np.float32)
    qb = np.asarray(inputs["qb"], np.float32)
    kw = np.asarray(inputs["kw"], np.float32)
    kb = np.asarray(inputs["kb"], np.float32)
    mw = np.asarray(inputs["mw"], np.float32)
    mb = np.asarray(inputs["mb"], np.float32)
    vw = np.asarray(inputs["vw"], np.float32)
    vb = np.asarray(inputs["vb"], np.float32)
    ww = np.asarray(inputs["ww"], np.float32)
    wb = np.asarray(inputs["wb"], np.float32)
    g = np.asarray(inputs["bn_gamma"], np.float32)
    be = np.asarray(inputs["bn_beta"], np.float32)
    rm = np.asarray(inputs["bn_rm"], np.float32)
    rv = np.asarray(inputs["bn_rv"], np.float32)

    qmT = np.zeros((C + 1, C + 1), np.float32)
    qmT[:C, :C] = qw.T
    qmT[C, :C] = qb
    qmT[:C, C] = mw[0]
    qmT[C, C] = mb[0]

    kT = np.concatenate([kw.T, kb[None, :]], axis=0)                # [65,64]

    vmT = np.zeros((C + 1, C + 1), np.float32)
    vmT[:C, :C] = vw.T
    vmT[C, :C] = vb
    vmT[:C, C] = mw[0]
    vmT[C, C] = mb[0]

    inv = g / np.sqrt(rv + EPS)
    wT = np.zeros((C + 1, C), np.float32)
    wT[:C, :] = (ww * inv[:, None]).T / N_CORES
    wT[C, :] = (wb * inv + be - rm * inv) / N_CORES

    common = {
        "x_ext": x_ext,
        "qmT": qmT.astype(BF16),
        "kT": kT.astype(BF16),
        "vmT": vmT.astype(BF16),
        "wT": wT.astype(BF16),
    }
    in_maps = []
    for ic in range(N_CORES):
        m = dict(common)
        m["xsl_ext"] = np.ascontiguousarray(x_ext[:, :, ic * SL:(ic + 1) * SL])
        in_maps.append(m)
    return in_maps


def kernel(**inputs):
    from concourse.bass_utils import run_bass_kernel_spmd

    nc = _get_program()
    in_maps = _prep_inputs(inputs)
    res = run_bass_kernel_spmd(nc, in_maps, core_ids=list(range(N_CORES)))
    y = np.zeros((B, C, N), np.float32)
    for r in res.results:
        y += r["y_part"]
    return y.reshape(B, C, H, W)


if __name__ == "__main__":
    rng = np.random.default_rng(0)
    ins = {
        "x": rng.standard_normal((B, C, H, W), dtype=np.float32),
        "qw": rng.standard_normal((C, C), dtype=np.float32) * 0.05,
        "qb": rng.standard_normal((C,), dtype=np.float32) * 0.05,
        "kw": rng.standard_normal((C, C), dtype=np.float32) * 0.05,
        "kb": rng.standard_normal((C,), dtype=np.float32) * 0.05,
        "mw": rng.standard_normal((1, C), dtype=np.float32) * 0.05,
        "mb": rng.standard_normal((1,), dtype=np.float32) * 0.05,
        "vw": rng.standard_normal((C, C), dtype=np.float32) * 0.05,
        "vb": rng.standard_normal((C,), dtype=np.float32) * 0.05,
        "ww": rng.standard_normal((C, C), dtype=np.float32) * 0.05,
        "wb": rng.standard_normal((C,), dtype=np.float32) * 0.05,
        "bn_gamma": np.ones((C,), np.float32),
        "bn_beta": np.zeros((C,), np.float32),
        "bn_rm": np.zeros((C,), np.float32),
        "bn_rv": np.ones((C,), np.float32),
    }
    out = kernel(**ins)
    print("kernel output", out.shape, out.dtype, np.abs(out).mean())



# revision 30
# speedup vs baseline: 1.0147x; 1.0147x over previous
"""Trainium2 Bass kernel for the non-local-attention block (nn_DNL_74234214744693).

Reference (B=4, C=64, H=W=64, N=4096):
    k = conv1x1(x,kw,kb); k_wh = k - mean_j(k)
    q = conv1x1(x,qw,qb)                      (q-whitening is a softmax no-op)
    qk[b,i,j] = sum_c k_wh[b,c,i] q[b,c,j]
    m  = conv1x1(x,mw,mb) -> [B,N];  mm[b,i,j] = m[b,i]*m[b,j]
    f  = softmax_j(qk) + softmax_b(mm)        (second softmax over BATCH)
    y  = einsum('bci,bij->bcj', v, f) + BN(conv1x1(x,ww,wb))

Key tricks (v3):
  * Associativity: qk = k_wh^T (qT^T x) = (qT @ k_wh)^T x.  The per-sample
    [65,512] matrix kq = qT @ k_wh becomes the qk stationary; the moving
    operand is x itself (fp8, resident).  The whole q conv and its PSUM->SBUF
    copies vanish.
  * e2 = exp(m_i m_j) has a rank-1 argument -> exp replaced by a degree-12
    polynomial evaluated as PE matmuls over a stacked (b,k) power basis
    (stored as (m/2)^k, fp8 hi/lo split, DoubleRow).  D = sum_b e2_b and
    h_b = e2_b - D/4 come from the same basis via row weights.
  * y2 mean-subtraction: y2 = S/4 + (v/4) @ g', g' = 4*f2 - 1 (fp8-safe).
    S = sum_n v via ones-matmul; added in the out-copy (Identity+bias).
  * fp8e4m3 + DoubleRow for qk / poly-h/D / y1 / g-y / wx (zero-padded
    stationaries; stride-0 second subtile plane for the moving operands).
  * softmax_j via constant shift exp(qk-7); Z free from accum_out; v1p=16v/Z.
    PSUM accumulates x16; out-copy applies /16 and adds S/4.
  * Decoupled PSUM pools: psQ (qk+exp rotation) never waits on the slow
    elementwise consumers that drain psHY (poly/g/y/convs).

Sharding: each of 8 cores owns a 512-row i-slice of the [N,N] maps for all 4
samples; host sums the 8 partial outputs (wx is pre-divided by 8).
"""

import functools

import numpy as np
import ml_dtypes

N_CORES = 8
B, C, H, W = 4, 64, 64, 64
N = H * W                 # 4096
SL = N // N_CORES         # 512
NIT = SL // 128           # 4
NITP = NIT // 2           # 2
NJ5 = N // 512            # 8
NJQ = N // 1024           # 4
EPS = 1e-5
SHIFT = 7.0
DEG = 12
KP = 16

# exp(t) ~= sum_k POLY_A[k] t^k on [-4, 4]; max abs err 4e-6.
POLY_A = [1.000000481756752, 0.9999888881522239, 0.49999706307401615,
          0.16668597667298232, 0.04166958451576583, 0.008323772405684203,
          0.0013878046435380107, 0.00020043162670676482,
          2.4992571180864735e-05, 2.550512749531329e-06,
          2.5846139980280564e-07, 3.4818470661121456e-08,
          2.8304950257085147e-09]

F8 = ml_dtypes.float8_e4m3
BF16 = ml_dtypes.bfloat16


def _build_program():
    import concourse.bass as bass
    import concourse.tile as tile
    from concourse import bacc, masks, mybir

    dt = mybir.dt
    AF = mybir.ActivationFunctionType
    ALU = mybir.AluOpType
    DR = mybir.MatmulPerfMode.DoubleRow

    nc = bacc.Bacc("TRN2", target_bir_lowering=False, debug=False,
                   enable_asserts=False, num_devices=1)

    # ---------------- DRAM I/O ----------------
    x_ext = nc.dram_tensor("x_ext", [B, C + 1, N], dt.bfloat16, kind="ExternalInput")
    xsl_ext = nc.dram_tensor("xsl_ext", [B, C + 1, SL], dt.bfloat16, kind="ExternalInput")
    x_f8 = nc.dram_tensor("x_f8", [B, 128, N], dt.float8e4, kind="ExternalInput")
    qTT = nc.dram_tensor("qTT", [C, C + 1], dt.bfloat16, kind="ExternalInput")
    kT = nc.dram_tensor("kT", [C + 1, C], dt.bfloat16, kind="ExternalInput")
    vmT = nc.dram_tensor("vmT", [C + 1, C + 1], dt.bfloat16, kind="ExternalInput")
    mwT = nc.dram_tensor("mwT", [C + 1, 1], dt.bfloat16, kind="ExternalInput")
    wT16 = nc.dram_tensor("wT16", [128, 2, 2, 2 * C], dt.float8e4, kind="ExternalInput")
    au_in = nc.dram_tensor("au_in", [64, 8], dt.float32, kind="ExternalInput")
    zeros8 = nc.dram_tensor("zeros8", [64, 8192], dt.float8e4, kind="ExternalInput")
    y_part = nc.dram_tensor("y_part", [B, C, N], dt.float32, kind="ExternalOutput")

    f8_, bf_, f32 = dt.float8e4, dt.bfloat16, dt.float32

    with tile.TileContext(nc) as tc:
        from contextlib import ExitStack

        with ExitStack() as top:
            consts = top.enter_context(tc.tile_pool(name="consts", bufs=1))
            p_xf8 = top.enter_context(tc.tile_pool(name="p_xf8", bufs=B))
            p_kq = top.enter_context(tc.tile_pool(name="p_kq", bufs=1))
            p_f1 = top.enter_context(tc.tile_pool(name="p_f1", bufs=B * NITP))
            p_vT = top.enter_context(tc.tile_pool(name="p_vT", bufs=B * NIT))
            p_vp = top.enter_context(tc.tile_pool(name="p_vp", bufs=2))
            p_us = top.enter_context(tc.tile_pool(name="p_us", bufs=1))
            p_vdr = top.enter_context(tc.tile_pool(name="p_vdr", bufs=1))
            p_tit = top.enter_context(tc.tile_pool(name="p_tit", bufs=NIT))
            p_sm = top.enter_context(tc.tile_pool(name="p_sm", bufs=48))
            p_out = top.enter_context(tc.tile_pool(name="p_out", bufs=2))
            psQ = top.enter_context(tc.tile_pool(name="psQ", bufs=2, space="PSUM"))
            psH = top.enter_context(tc.tile_pool(name="psH", bufs=4, space="PSUM"))

            # ---------------- consts ----------------
            sb_qTT = consts.tile([C, C + 1], bf_)
            sb_kT = consts.tile([C + 1, C], bf_)
            sb_vmT = consts.tile([C + 1, C + 1], bf_)
            sb_mwT = consts.tile([C + 1, 1], bf_)
            sb_wT = consts.tile([128, 2, 2, 2 * C], f8_)
            sb_au = consts.tile([64, 8], f32)
            ident = consts.tile([128, 128], f32)
            bias_t = consts.tile([128, 1], f32)
            ones_t = consts.tile([128, 1], bf_)
            nc.scalar.dma_start(sb_qTT, qTT.ap())
            nc.scalar.dma_start(sb_kT, kT.ap())
            nc.scalar.dma_start(sb_vmT, vmT.ap())
            nc.scalar.dma_start(sb_mwT, mwT.ap())
            nc.scalar.dma_start(sb_wT, wT16.ap())
            nc.scalar.dma_start(sb_au, au_in.ap())
            masks.make_identity(nc, ident[:])
            nc.vector.memset(bias_t, -SHIFT)
            nc.vector.memset(ones_t, 1.0)

            # ---------------- persistent tiles ----------------
            xf8 = [p_xf8.tile([128, N], f8_, name=f"xf8_{b}", tag="xf8") for b in range(B)]
            kq_dr = p_kq.tile([128, 2, B * SL], f8_)
            f1 = [[p_f1.tile([128, 2, N], f8_, name=f"f1_{b}_{p}", tag="f1")
                   for p in range(NITP)] for b in range(B)]
            v_T = [[p_vT.tile([128, C], bf_, name=f"v_T{b}_{i}", tag="v_T")
                    for i in range(NIT)] for b in range(B)]
            v1p_all = p_vp.tile([128, 2, B * NITP * 2 * C], f8_)
            vT4_all = p_vp.tile([128, 2, B * NITP * 2 * C], f8_)
            u_all = p_us.tile([128, 2, (B + 1) * SL], f8_)
            u_st = [u_all[:, :, x * SL:(x + 1) * SL] for x in range(B + 1)]
            v_dr = p_vdr.tile([128, 2, N], f8_)
            t_it = [p_tit.tile([128, B, KP], f32, name=f"t_it{i}", tag="t_it")
                    for i in range(NIT)]
            sq = [p_sm.tile([128, 1], f32, name=f"sq{p}", tag="sq") for p in range(2)]

            # ---------------- zero-fill DR pads ----------------
            z = zeros8.ap()
            nc.sync.dma_start(v_dr[64:128, 0, :], z[:, 0:N])
            nc.sync.dma_start(v_dr[64:128, 1, :], z[:, 0:N])
            nc.sync.dma_start(kq_dr[65:128, 0, :], z[0:63, 0:B * SL])
            nc.sync.dma_start(kq_dr[0:64, 1, :], z[:, 0:B * SL])
            nc.sync.dma_start(kq_dr[64:128, 1, :], z[:, 0:B * SL])
            for t2 in (v1p_all, vT4_all):
                nc.sync.dma_start(t2[0:64, 0, :], z[:, 0:B * NITP * 2 * C])
                nc.sync.dma_start(t2[0:64, 1, :], z[:, 0:B * NITP * 2 * C])
                nc.sync.dma_start(t2[64:128, 0, :], z[:, 0:B * NITP * 2 * C])
                nc.sync.dma_start(t2[64:128, 1, :], z[:, 0:B * NITP * 2 * C])
            nc.sync.dma_start(u_all[64:128, 0, :], z[:, 0:(B + 1) * SL])
            nc.sync.dma_start(u_all[64:128, 1, :], z[:, 0:(B + 1) * SL])

            def vst(all_t, bb, itp):
                o = (bb * NITP + itp) * 2 * C
                return all_t[:, :, o:o + 2 * C]

            def vwr(all_t, bb, itp, s_):
                o = (bb * NITP + itp) * 2 * C + (bb % 2) * C
                return all_t[:, s_, o:o + C]

            def dr_mov(tile2d, jsl):
                return tile2d[:, jsl].unsqueeze(1).to_broadcast(
                    [128, 2, jsl.stop - jsl.start])

            with ExitStack() as p01:
                p_vlad = p01.enter_context(tc.tile_pool(name="p_vlad", bufs=1))
                p_x = p01.enter_context(tc.tile_pool(name="p_x", bufs=2))
                p_xsl = p01.enter_context(tc.tile_pool(name="p_xsl", bufs=2))
                p_kwh = p01.enter_context(tc.tile_pool(name="p_kwh", bufs=2))
                v_lad = p_vlad.tile([128, 32, B, KP], f32)

                # ---------------- P0: per-sample convs ----------------
                for b in range(B):
                    x_sb = p_x.tile([C + 1, N], bf_, name=f"x_sb{b}", tag="x_sb")
                    xsl_sb = p_xsl.tile([C + 1, SL], bf_, name=f"xsl{b}", tag="xsl")
                    nc.sync.dma_start(x_sb, x_ext.ap()[b])
                    nc.sync.dma_start(xsl_sb, xsl_ext.ap()[b])
                    nc.sync.dma_start(xf8[b], x_f8.ap()[b])

                    # xu = mean_j x via DVE 2x tensor_scalar with accum
                    # (throwaway bf16 output into not-yet-written f1 space)
                    xu_f = p_sm.tile([C + 1, 1], f32, name=f"xu_f{b}", tag="xu_f")
                    xu_bf = p_sm.tile([C + 1, 1], bf_, name=f"xu_bf{b}", tag="xu_bf")
                    xuh = [p_sm.tile([C + 1, 1], f32, name=f"xuh{_h}", tag="xuh")
                           for _h in range(2)]
                    xud = f1[3][1][:, :, :].bitcast(bf_)
                    for _h in range(2):
                        nc.vector.tensor_scalar(xud[0:C + 1, _h, :],
                                                x_sb[:, _h * 2048:(_h + 1) * 2048],
                                                1.0 / N, 0.0, op0=ALU.mult,
                                                op1=ALU.add, accum_out=xuh[_h])
                    nc.vector.tensor_tensor(xu_f, xuh[0], xuh[1], op=ALU.add)
                    nc.vector.tensor_copy(xu_bf, xu_f)

                    # vm/mT/ku/S gen (psH), k conv in a second gen
                    ps = psH.tile([128, 512], f32, name=f"vm{b}", tag="psH")
                    for it in range(NIT):
                        nc.tensor.matmul(ps[:, it * 65:(it + 1) * 65],
                                         xsl_sb[:, it * 128:(it + 1) * 128],
                                         sb_vmT, start=True, stop=True)
                    for t in range(32):
                        nc.tensor.matmul(ps[:, 264 + t:265 + t],
                                         x_sb[:, t * 128:(t + 1) * 128],
                                         sb_mwT, start=True, stop=True)
                    nc.tensor.matmul(ps[0:C, 300:301], sb_kT, xu_bf,
                                     start=True, stop=True)
                    psk2 = psH.tile([128, 512], f32, name=f"kc{b}", tag="psH")
                    nc.tensor.matmul(psk2[0:C, :], sb_kT, xsl_sb,
                                     start=True, stop=True)

                    for it in range(NIT):
                        cs = slice(it * 65, it * 65 + C)
                        nc.scalar.copy(v_T[b][it], ps[:, cs])
                        nc.gpsimd.tensor_scalar_mul(vwr(vT4_all, b, it // 2, it % 2),
                                                    v_T[b][it], 4.0)
                        nc.scalar.mul(t_it[it][:, b, 1:2],
                                      ps[:, it * 65 + C:it * 65 + C + 1], 0.5)
                    nc.scalar.mul(v_lad[:, :, b, 1], ps[:, 264:296], 0.5)

                    negku = p_sm.tile([C, 1], f32, name=f"negku{b}", tag="negku")
                    nc.vector.tensor_scalar_mul(negku, ps[0:C, 300:301], -1.0)
                    kwh_sb = p_kwh.tile([C, SL], bf_, name=f"kwh{b}", tag="kwh")
                    nc.scalar.activation(kwh_sb, psk2[0:C, :], AF.Identity,
                                         bias=negku[:], scale=1.0)

                    # S = sum_n v (ones matmul, accumulated over it)
                    po = (b % 2) * 64
                    for it in range(NIT):
                        nc.tensor.matmul(ps[po:po + C, 301:302], v_T[b][it], ones_t,
                                         start=(it == 0), stop=(it == NIT - 1))
                    nc.scalar.mul(sq[b // 2][po:po + C, :], ps[po:po + C, 301:302], 0.25)

                    # kq = qT @ k_wh  -> fp8 DR stationary for the qk matmuls
                    ps2 = psH.tile([128, 512], f32, name=f"kq{b}", tag="psH")
                    nc.tensor.matmul(ps2[0:C + 1, 0:SL], sb_qTT, kwh_sb,
                                     start=True, stop=True)
                    nc.scalar.copy(kq_dr[0:C + 1, 0, b * SL:(b + 1) * SL],
                                   ps2[0:C + 1, 0:SL])

                # ---------------- P1: power ladders + transposes ----------------
                nc.gpsimd.memset(v_lad[:, :, :, 0:1], 1.0)
                for it in range(NIT):
                    nc.gpsimd.memset(t_it[it][:, :, 0:1], 1.0)
                for k in range(2, DEG + 1):
                    nc.gpsimd.tensor_tensor(v_lad[:, :, :, k], v_lad[:, :, :, k - 1],
                                            v_lad[:, :, :, 1], op=ALU.mult)
                for it in range(NIT):
                    for k in range(2, DEG + 1):
                        nc.gpsimd.tensor_tensor(t_it[it][:, :, k], t_it[it][:, :, k - 1],
                                                t_it[it][:, :, 1], op=ALU.mult)

                # V transposes: [128 j, (b,k)] -> [(b,k), j] with fp8 hi/lo
                for gen in range(8):
                    psv = psH.tile([128, 512], f32, name=f"vt{gen}", tag="psH")
                    for j4 in range(4):
                        jt = gen * 4 + j4
                        nc.tensor.transpose(psv[0:64, j4 * 128:(j4 + 1) * 128],
                                            v_lad[:, jt], ident)
                    csl = slice(gen * 512, (gen + 1) * 512)
                    nc.scalar.copy(v_dr[0:64, 0, csl], psv[0:64, :])
                    nc.vector.tensor_tensor(v_dr[0:64, 1, csl], psv[0:64, :],
                                            v_dr[0:64, 0, csl], op=ALU.subtract)

                # U transposes + stationaries
                psu = psH.tile([128, 512], f32, name="ut", tag="psH")
                for it in range(NIT):
                    nc.tensor.transpose(psu[0:64, it * 128:(it + 1) * 128],
                                        t_it[it], ident)
                for x in range(B + 1):
                    auc = sb_au[:, x:x + 1]
                    nc.scalar.mul(u_st[x][0:64, 0, :], psu[0:64, 0:SL], auc[0:64])
                    nc.vector.scalar_tensor_tensor(u_st[x][0:64, 1, :], psu[0:64, 0:SL],
                                                   auc[0:64], u_st[x][0:64, 0, :],
                                                   op0=ALU.mult, op1=ALU.subtract)

            # ---------------- P2: qk/exp + poly/g + y ----------------
            # (pools opened after P0/P1 scope closed -> reuse v_lad space)
            p_g0 = top.enter_context(tc.tile_pool(name="p_g0", bufs=16))
            p_g1 = top.enter_context(tc.tile_pool(name="p_g1", bufs=16))
            p_rr = top.enter_context(tc.tile_pool(name="p_rr", bufs=2))
            g_tiles = {}
            out_tiles = {}

            def hd_round(jq, rit):
                itp, s = rit // 2, rit % 2
                i_sl = slice(rit * 128, (rit + 1) * 128)
                rr = p_rr.tile([128, 1024], f32, name=f"rr{jq}_{rit}", tag="rr")
                for hh in range(2):
                    jsl = slice(jq * 1024 + hh * 512, jq * 1024 + (hh + 1) * 512)
                    psd = psH.tile([128, 512], f32, name=f"hdD{jq}_{rit}_{hh}", tag="psH")
                    nc.tensor.matmul(psd, u_st[B][:, :, i_sl], v_dr[:, :, jsl],
                                     start=True, stop=True, perf_mode=DR)
                    nc.vector.reciprocal_approx_fast(rr[:, hh * 512:(hh + 1) * 512], psd)
                for bb in range(B):
                    key = (bb, itp, jq)
                    if key not in g_tiles:
                        pool = p_g0 if bb < 2 else p_g1
                        g_tiles[key] = pool.tile([128, 2, 1024], f8_,
                                                 name=f"g{bb}_{itp}_{jq}", tag="g")
                    for hh in range(2):
                        jsl = slice(jq * 1024 + hh * 512, jq * 1024 + (hh + 1) * 512)
                        psh_ = psH.tile([128, 512], f32,
                                        name=f"hd{bb}_{jq}_{rit}_{hh}", tag="psH")
                        nc.tensor.matmul(psh_, u_st[bb][:, :, i_sl], v_dr[:, :, jsl],
                                         start=True, stop=True, perf_mode=DR)
                        nc.vector.tensor_tensor(
                            g_tiles[key][:, s, hh * 512:(hh + 1) * 512],
                            psh_, rr[:, hh * 512:(hh + 1) * 512], op=ALU.mult)

            def y_round(pair, j5, tail=False):
                jsl = slice(j5 * 512, (j5 + 1) * 512)
                jq, jh = j5 // 2, j5 % 2
                if tail:
                    ps = psQ.tile([128, 1024], f32, name=f"y{pair}_{j5}", tag="psQ")
                else:
                    ps = psH.tile([128, 512], f32, name=f"y{pair}_{j5}", tag="psH")
                reg = ps[:, 0:512]
                for i in range(2):
                    bb = 2 * pair + i
                    nc.tensor.matmul(reg, sb_wT[:, :, i, :], dr_mov(xf8[bb], jsl),
                                     start=(i == 0), stop=False, perf_mode=DR)
                    for itp in range(NITP):
                        nc.tensor.matmul(reg, vst(v1p_all, bb, itp),
                                         f1[bb][itp][:, :, jsl],
                                         start=False, stop=False, perf_mode=DR)
                        nc.tensor.matmul(
                            reg, vst(vT4_all, bb, itp),
                            g_tiles[(bb, itp, jq)][:, :, jh * 512:(jh + 1) * 512],
                            start=False,
                            stop=(i == 1 and itp == NITP - 1), perf_mode=DR)
                if (pair, jq) not in out_tiles:
                    out_tiles[(pair, jq)] = p_out.tile(
                        [128, 1024], f32, name=f"o{pair}_{jq}", tag="out_sb")
                out_sb = out_tiles[(pair, jq)]
                osl = slice(jh * 512, (jh + 1) * 512)
                nc.scalar.activation(out_sb[:, osl], ps[:, 0:512], AF.Identity,
                                     bias=sq[pair][:], scale=1.0 / 16.0)
                if jh == 1:
                    for i in range(2):
                        bb = 2 * pair + i
                        nc.sync.dma_start(y_part.ap()[bb][:, jq * 1024:(jq + 1) * 1024],
                                          out_sb[i * 64:i * 64 + C, :])

            # hd pacing: front 2/slot then 1/slot -> jq3 done by slot 13
            hd_q = [(jq, rit) for jq in range(NJQ) for rit in range(NIT)]
            HD_PACE = [1] * 16
            y_done = 0
            slot = 0
            for b in range(B):
                for it in range(NIT):
                    itp, s = it // 2, it % 2
                    zp = [p_sm.tile([128, 1], f32, name=f"zp{_h}", tag="zp")
                          for _h in range(4)]
                    for quar in range(4):
                        psk = psQ.tile([128, 1024], f32, name=f"qk{b}_{it}_{quar}",
                                       tag="psQ")
                        for q2 in range(2):
                            j0 = quar * 1024 + q2 * 512
                            nc.tensor.matmul(
                                psk[:, q2 * 512:(q2 + 1) * 512],
                                kq_dr[:, :, b * SL + it * 128:b * SL + (it + 1) * 128],
                                dr_mov(xf8[b], slice(j0, j0 + 512)),
                                start=True, stop=True, perf_mode=DR)
                        nc.scalar.activation(
                            f1[b][itp][:, s, quar * 1024:(quar + 1) * 1024],
                            psk, AF.Exp, bias=bias_t[:], accum_out=zp[quar])
                    z1a = p_sm.tile([128, 1], f32, name="z1a", tag="z1a")
                    z1b = p_sm.tile([128, 1], f32, name="z1b", tag="z1b")
                    z1 = p_sm.tile([128, 1], f32, name="z1", tag="z1")
                    rz = p_sm.tile([128, 1], f32, name="rz", tag="rz")
                    nc.gpsimd.tensor_tensor(z1a, zp[0], zp[1], op=ALU.add)
                    nc.gpsimd.tensor_tensor(z1b, zp[2], zp[3], op=ALU.add)
                    nc.gpsimd.tensor_tensor(z1, z1a, z1b, op=ALU.add)
                    nc.vector.reciprocal_approx_fast(rz, z1)
                    nc.gpsimd.tensor_scalar(vwr(v1p_all, b, itp, s), v_T[b][it],
                                            scalar1=rz, scalar2=16.0,
                                            op0=ALU.mult, op1=ALU.mult)

                    if slot >= 8 and y_done < 6:
                        y_round(0, y_done)
                        y_done += 1
                    for _ in range(HD_PACE[slot]):
                        if hd_q:
                            hd_round(*hd_q.pop(0))
                    slot += 1

            while hd_q:
                hd_round(*hd_q.pop(0))
            while y_done < NJ5:
                y_round(0, y_done, tail=True)
                y_done += 1
            for j5 in range(NJ5):
                y_round(1, j5, tail=True)

    nc.compile()
    return nc


@functools.lru_cache(maxsize=1)
def _get_program():
    return _build_program()


def _prep_inputs(inputs):
    x = np.asarray(inputs["x"], np.float32).reshape(B, C, N)
    ones = np.ones((B, 1, N), np.float32)
    x65 = np.concatenate([x, ones], axis=1)                         # [B,65,N]
    x_ext = x65.astype(BF16)
    x_f8 = np.zeros((B, 128, N), F8)
    x_f8[:, :C + 1] = x65.astype(F8)

    qw = np.asarray(inputs["qw"], np.float32)
    qb = np.asarray(inputs["qb"], np.float32)
    kw = np.asarray(inputs["kw"], np.float32)
    kb = np.asarray(inputs["kb"], np.float32)
    mw = np.asarray(inputs["mw"], np.float32)
    mb = np.asarray(inputs["mb"], np.float32)
    vw = np.asarray(inputs["vw"], np.float32)
    vb = np.asarray(inputs["vb"], np.float32)
    ww = np.asarray(inputs["ww"], np.float32)
    wb = np.asarray(inputs["wb"], np.float32)
    g = np.asarray(inputs["bn_gamma"], np.float32)
    be = np.asarray(inputs["bn_beta"], np.float32)
    rm = np.asarray(inputs["bn_rm"], np.float32)
    rv = np.asarray(inputs["bn_rv"], np.float32)

    qTTa = np.concatenate([qw, qb[:, None]], axis=1)                # [64,65]
    kTa = np.concatenate([kw.T, kb[None, :]], axis=0)               # [65,64]

    vmT = np.zeros((C + 1, C + 1), np.float32)
    vmT[:C, :C] = vw.T
    vmT[C, :C] = vb
    vmT[:C, C] = mw[0]
    vmT[C, C] = mb[0]

    mwT = np.concatenate([mw[0][:, None], mb[:, None]], axis=0)     # [65,1]

    inv = g / np.sqrt(rv + EPS)
    wT_bn = np.zeros((C + 1, C), np.float32)
    wT_bn[:C, :] = (ww * inv[:, None]).T
    wT_bn[C, :] = wb * inv + be - rm * inv
    wT16 = np.zeros((128, 2, 2, 2 * C), np.float32)
    for i in range(2):
        wT16[0:C + 1, 0, i, i * C:(i + 1) * C] = (16.0 / N_CORES) * wT_bn

    # au rows ordered (b'*16 + k): weights for the (m/2)^k power basis.
    au = np.zeros((64, 8), np.float32)
    for bp in range(B):
        for k in range(DEG + 1):
            a4 = POLY_A[k] * (4.0 ** k)
            for bt in range(B):
                au[bp * KP + k, bt] = a4 * (0.75 if bp == bt else -0.25)
            au[bp * KP + k, 4] = a4 * 0.25
    zeros8_a = np.zeros((64, 8192), F8)

    common = {
        "x_ext": x_ext,
        "x_f8": x_f8,
        "qTT": qTTa.astype(BF16),
        "kT": kTa.astype(BF16),
        "vmT": vmT.astype(BF16),
        "mwT": mwT.astype(BF16),
        "wT16": wT16.astype(F8),
        "au_in": au,
        "zeros8": zeros8_a,
    }
    in_maps = []
    for ic in range(N_CORES):
        m = dict(common)
        m["xsl_ext"] = np.ascontiguousarray(x_ext[:, :, ic * SL:(ic + 1) * SL])
        in_maps.append(m)
    return in_maps


def kernel(**inputs):
    from concourse.bass_utils import run_bass_kernel_spmd

    nc = _get_program()
    in_maps = _prep_inputs(inputs)
    res = run_bass_kernel_spmd(nc, in_maps, core_ids=list(range(N_CORES)))
    y = np.zeros((B, C, N), np.float32)
    for r in res.results:
        y += r["y_part"]
    return y.reshape(B, C, H, W)


if __name__ == "__main__":
    rng = np.random.default_rng(0)
    ins = {
        "x": rng.standard_normal((B, C, H, W), dtype=np.float32),
        "qw": rng.standard_normal((C, C), dtype=np.float32) * 0.05,
        "qb": rng.standard_normal((C,), dtype=np.float32) * 0.05,
        "kw": rng.standard_normal((C, C), dtype=np.float32) * 0.05,
        "kb": rng.standard_normal((C,), dtype=np.float32) * 0.05,
        "mw": rng.standard_normal((1, C), dtype=np.float32) * 0.05,
        "mb": rng.standard_normal((1,), dtype=np.float32) * 0.05,
        "vw": rng.standard_normal((C, C), dtype=np.float32) * 0.05,
        "vb": rng.standard_normal((C,), dtype=np.float32) * 0.05,
        "ww": rng.standard_normal((C, C), dtype=np.float32) * 0.05,
        "wb": rng.standard_normal((C,), dtype=np.float32) * 0.05,
        "bn_gamma": np.ones((C,), np.float32),
        "bn_beta": np.zeros((C,), np.float32),
        "bn_rm": np.zeros((C,), np.float32),
        "bn_rv": np.ones((C,), np.float32),
    }
    out = kernel(**ins)
    print("kernel output", out.shape, out.dtype, np.abs(out).mean())


# revision 38
# speedup vs baseline: 1.1140x; 1.0979x over previous
"""Trainium2 Bass kernel for the non-local-attention block (nn_DNL_74234214744693).

Reference (B=4, C=64, H=W=64, N=4096):
    k = conv1x1(x,kw,kb); k_wh = k - mean_j(k)
    q = conv1x1(x,qw,qb)                      (q-whitening is a softmax no-op)
    qk[b,i,j] = sum_c k_wh[b,c,i] q[b,c,j]
    m  = conv1x1(x,mw,mb) -> [B,N];  mm[b,i,j] = m[b,i]*m[b,j]
    f  = softmax_j(qk) + softmax_b(mm)        (second softmax over BATCH)
    y  = einsum('bci,bij->bcj', v, f) + BN(conv1x1(x,ww,wb))

Key tricks (v3):
  * Associativity: qk = k_wh^T (qT^T x) = (qT @ k_wh)^T x.  The per-sample
    [65,512] matrix kq = qT @ k_wh becomes the qk stationary; the moving
    operand is x itself (fp8, resident).  The whole q conv and its PSUM->SBUF
    copies vanish.
  * e2 = exp(m_i m_j) has a rank-1 argument -> exp replaced by a degree-12
    polynomial evaluated as PE matmuls over a stacked (b,k) power basis
    (stored as (m/2)^k, fp8 hi/lo split, DoubleRow).  D = sum_b e2_b and
    h_b = e2_b - D/4 come from the same basis via row weights.
  * y2 mean-subtraction: y2 = S/4 + (v/4) @ g', g' = 4*f2 - 1 (fp8-safe).
    S = sum_n v via ones-matmul; added in the out-copy (Identity+bias).
  * fp8e4m3 + DoubleRow for qk / poly-h/D / y1 / g-y / wx (zero-padded
    stationaries; stride-0 second subtile plane for the moving operands).
  * softmax_j via constant shift exp(qk-7); Z free from accum_out; v1p=16v/Z.
    PSUM accumulates x16; out-copy applies /16 and adds S/4.
  * Decoupled PSUM pools: psQ (qk+exp rotation) never waits on the slow
    elementwise consumers that drain psHY (poly/g/y/convs).

Sharding: each of 8 cores owns a 512-row i-slice of the [N,N] maps for all 4
samples; host sums the 8 partial outputs (wx is pre-divided by 8).
"""

import functools

import numpy as np
import ml_dtypes

N_CORES = 8
B, C, H, W = 4, 64, 64, 64
N = H * W                 # 4096
SL = N // N_CORES         # 512
NIT = SL // 128           # 4
NITP = NIT // 2           # 2
NJ5 = N // 512            # 8
NJQ = N // 1024           # 4
EPS = 1e-5
SHIFT = 7.0
DEG = 12
KP = 16

# exp(t) ~= sum_k POLY_A[k] t^k on [-4, 4]; max abs err 4e-6.
POLY_A = [1.000000481756752, 0.9999888881522239, 0.49999706307401615,
          0.16668597667298232, 0.04166958451576583, 0.008323772405684203,
          0.0013878046435380107, 0.00020043162670676482,
          2.4992571180864735e-05, 2.550512749531329e-06,
          2.5846139980280564e-07, 3.4818470661121456e-08,
          2.8304950257085147e-09]

F8 = ml_dtypes.float8_e4m3
BF16 = ml_dtypes.bfloat16


def _build_program():
    import concourse.bass as bass
    import concourse.tile as tile
    from concourse import bacc, masks, mybir

    dt = mybir.dt
    AF = mybir.ActivationFunctionType
    ALU = mybir.AluOpType
    DR = mybir.MatmulPerfMode.DoubleRow

    nc = bacc.Bacc("TRN2", target_bir_lowering=False, debug=False,
                   enable_asserts=False, num_devices=1)

    # ---------------- DRAM I/O ----------------
    x_ext = nc.dram_tensor("x_ext", [B, C + 1, N], dt.bfloat16, kind="ExternalInput")
    xsl_ext = nc.dram_tensor("xsl_ext", [B, C + 1, SL], dt.bfloat16, kind="ExternalInput")
    x_f8 = nc.dram_tensor("x_f8", [B, 128, N], dt.float8e4, kind="ExternalInput")
    qTT = nc.dram_tensor("qTT", [C, C + 1], dt.bfloat16, kind="ExternalInput")
    kT = nc.dram_tensor("kT", [C + 1, C], dt.bfloat16, kind="ExternalInput")
    vmT = nc.dram_tensor("vmT", [C + 1, C + 1], dt.bfloat16, kind="ExternalInput")
    mwT = nc.dram_tensor("mwT", [C + 1, 1], dt.bfloat16, kind="ExternalInput")
    wT16 = nc.dram_tensor("wT16", [128, 2, 2, 2 * C], dt.float8e4, kind="ExternalInput")
    au_in = nc.dram_tensor("au_in", [64, 8], dt.float32, kind="ExternalInput")
    zeros8 = nc.dram_tensor("zeros8", [64, 8192], dt.float8e4, kind="ExternalInput")
    y_part = nc.dram_tensor("y_part", [B, C, N], dt.float32, kind="ExternalOutput")

    f8_, bf_, f32 = dt.float8e4, dt.bfloat16, dt.float32

    with tile.TileContext(nc) as tc:
        from contextlib import ExitStack

        with ExitStack() as top:
            consts = top.enter_context(tc.tile_pool(name="consts", bufs=1))
            p_xf8 = top.enter_context(tc.tile_pool(name="p_xf8", bufs=B))
            p_kq = top.enter_context(tc.tile_pool(name="p_kq", bufs=1))
            p_f1 = top.enter_context(tc.tile_pool(name="p_f1", bufs=B * NITP))
            p_vT = top.enter_context(tc.tile_pool(name="p_vT", bufs=B * NIT))
            p_vp = top.enter_context(tc.tile_pool(name="p_vp", bufs=2))
            p_us = top.enter_context(tc.tile_pool(name="p_us", bufs=1))
            p_vdr = top.enter_context(tc.tile_pool(name="p_vdr", bufs=1))
            p_tit = top.enter_context(tc.tile_pool(name="p_tit", bufs=NIT))
            p_sm = top.enter_context(tc.tile_pool(name="p_sm", bufs=48))
            p_zp = top.enter_context(tc.tile_pool(name="p_zp", bufs=24))
            p_out = top.enter_context(tc.tile_pool(name="p_out", bufs=2))
            psQ = top.enter_context(tc.tile_pool(name="psQ", bufs=2, space="PSUM"))
            psH = top.enter_context(tc.tile_pool(name="psH", bufs=2, space="PSUM"))

            # ---------------- consts ----------------
            sb_qTT = consts.tile([C, C + 1], bf_)
            sb_kT = consts.tile([C + 1, C], bf_)
            sb_vmT = consts.tile([C + 1, C + 1], bf_)
            sb_mwT = consts.tile([C + 1, 1], bf_)
            sb_wT = consts.tile([128, 2, 2, 2 * C], f8_)
            sb_au = consts.tile([64, 8], f32)
            ident = consts.tile([128, 128], f32)
            bias_t = consts.tile([128, 1], f32)
            ones_t = consts.tile([128, 1], bf_)
            nc.scalar.dma_start(sb_qTT, qTT.ap())
            nc.scalar.dma_start(sb_kT, kT.ap())
            nc.scalar.dma_start(sb_vmT, vmT.ap())
            nc.scalar.dma_start(sb_mwT, mwT.ap())
            nc.scalar.dma_start(sb_wT, wT16.ap())
            nc.scalar.dma_start(sb_au, au_in.ap())
            masks.make_identity(nc, ident[:])
            nc.vector.memset(bias_t, -SHIFT)
            nc.vector.memset(ones_t, 1.0)

            # ---------------- persistent tiles ----------------
            xf8 = [p_xf8.tile([128, N], f8_, name=f"xf8_{b}", tag="xf8") for b in range(B)]
            kq_dr = p_kq.tile([128, 2, B * SL], f8_)
            f1 = [[p_f1.tile([128, 2, N], f8_, name=f"f1_{b}_{p}", tag="f1")
                   for p in range(NITP)] for b in range(B)]
            v_T = [[p_vT.tile([128, C], bf_, name=f"v_T{b}_{i}", tag="v_T")
                    for i in range(NIT)] for b in range(B)]
            v1p_all = p_vp.tile([128, 2, B * NITP * 2 * C], f8_)
            vT4_all = p_vp.tile([128, 2, B * NITP * 2 * C], f8_)
            u_all = p_us.tile([128, 2, (B + 1) * SL], f8_)
            u_st = [u_all[:, :, x * SL:(x + 1) * SL] for x in range(B + 1)]
            v_dr = p_vdr.tile([128, 2, N], f8_)
            t_it = [p_tit.tile([128, B, KP], f32, name=f"t_it{i}", tag="t_it")
                    for i in range(NIT)]
            sq = [p_sm.tile([128, 1], f32, name=f"sq{p}", tag="sq") for p in range(2)]

            # ---------------- zero-fill DR pads ----------------
            z = zeros8.ap()
            nc.sync.dma_start(v_dr[64:128, 0, :], z[:, 0:N])
            nc.sync.dma_start(v_dr[64:128, 1, :], z[:, 0:N])
            nc.sync.dma_start(kq_dr[65:128, 0, :], z[0:63, 0:B * SL])
            nc.sync.dma_start(kq_dr[0:64, 1, :], z[:, 0:B * SL])
            nc.sync.dma_start(kq_dr[64:128, 1, :], z[:, 0:B * SL])
            for t2 in (v1p_all, vT4_all):
                nc.sync.dma_start(t2[0:64, 0, :], z[:, 0:B * NITP * 2 * C])
                nc.sync.dma_start(t2[0:64, 1, :], z[:, 0:B * NITP * 2 * C])
                nc.sync.dma_start(t2[64:128, 0, :], z[:, 0:B * NITP * 2 * C])
                nc.sync.dma_start(t2[64:128, 1, :], z[:, 0:B * NITP * 2 * C])
            nc.sync.dma_start(u_all[64:128, 0, :], z[:, 0:(B + 1) * SL])
            nc.sync.dma_start(u_all[64:128, 1, :], z[:, 0:(B + 1) * SL])

            def vst(all_t, bb, itp):
                o = (bb * NITP + itp) * 2 * C
                return all_t[:, :, o:o + 2 * C]

            def vwr(all_t, bb, itp, s_):
                o = (bb * NITP + itp) * 2 * C + (bb % 2) * C
                return all_t[:, s_, o:o + C]

            def dr_mov(tile2d, jsl):
                return tile2d[:, jsl].unsqueeze(1).to_broadcast(
                    [128, 2, jsl.stop - jsl.start])

            with ExitStack() as p01:
                p_vlad = p01.enter_context(tc.tile_pool(name="p_vlad", bufs=1))
                p_x = p01.enter_context(tc.tile_pool(name="p_x", bufs=2))
                p_xsl = p01.enter_context(tc.tile_pool(name="p_xsl", bufs=2))
                p_kwh = p01.enter_context(tc.tile_pool(name="p_kwh", bufs=2))
                v_lad = p_vlad.tile([128, 32, B, KP], f32)

                # ---------------- P0: per-sample convs ----------------
                for b in range(B):
                    x_sb = p_x.tile([C + 1, N], bf_, name=f"x_sb{b}", tag="x_sb")
                    xsl_sb = p_xsl.tile([C + 1, SL], bf_, name=f"xsl{b}", tag="xsl")
                    nc.sync.dma_start(x_sb, x_ext.ap()[b])
                    nc.sync.dma_start(xsl_sb, xsl_ext.ap()[b])
                    nc.sync.dma_start(xf8[b], x_f8.ap()[b])

                    # xu = mean_j x via DVE 2x tensor_scalar with accum
                    # (throwaway bf16 output into not-yet-written f1 space)
                    xu_f = p_sm.tile([C + 1, 1], f32, name=f"xu_f{b}", tag="xu_f")
                    xu_bf = p_sm.tile([C + 1, 1], bf_, name=f"xu_bf{b}", tag="xu_bf")
                    xuh = [p_sm.tile([C + 1, 1], f32, name=f"xuh{_h}", tag="xuh")
                           for _h in range(2)]
                    xud = f1[3][1][:, :, :].bitcast(bf_)
                    xeng = nc.vector
                    for _h in range(2):
                        xeng.tensor_scalar(xud[0:C + 1, _h, :],
                                           x_sb[:, _h * 2048:(_h + 1) * 2048],
                                           1.0 / N, 0.0, op0=ALU.mult,
                                           op1=ALU.add, accum_out=xuh[_h])
                    xeng.tensor_tensor(xu_f, xuh[0], xuh[1], op=ALU.add)
                    xeng.tensor_copy(xu_bf, xu_f)

                    # vm/mT/ku/S gen (psH), k conv in a second gen
                    ps = psH.tile([128, 512], f32, name=f"vm{b}", tag="psH")
                    for it in range(NIT):
                        nc.tensor.matmul(ps[:, it * 65:(it + 1) * 65],
                                         xsl_sb[:, it * 128:(it + 1) * 128],
                                         sb_vmT, start=True, stop=True)
                    for t in range(32):
                        nc.tensor.matmul(ps[:, 264 + t:265 + t],
                                         x_sb[:, t * 128:(t + 1) * 128],
                                         sb_mwT, start=True, stop=True)
                    nc.tensor.matmul(ps[0:C, 300:301], sb_kT, xu_bf,
                                     start=True, stop=True)
                    psk2 = psH.tile([128, 512], f32, name=f"kc{b}", tag="psH")
                    nc.tensor.matmul(psk2[0:C, :], sb_kT, xsl_sb,
                                     start=True, stop=True)

                    for it in range(NIT):
                        cs = slice(it * 65, it * 65 + C)
                        nc.vector.tensor_copy(v_T[b][it], ps[:, cs])
                        nc.gpsimd.tensor_scalar_mul(vwr(vT4_all, b, it // 2, it % 2),
                                                    v_T[b][it], 4.0)
                        nc.vector.tensor_scalar_mul(t_it[it][:, b, 1:2],
                                                    ps[:, it * 65 + C:it * 65 + C + 1], 0.5)
                    nc.vector.tensor_scalar_mul(v_lad[:, :, b, 1], ps[:, 264:296], 0.5)

                    negku = p_sm.tile([C, 1], f32, name=f"negku{b}", tag="negku")
                    nc.vector.tensor_scalar_mul(negku, ps[0:C, 300:301], -1.0)
                    kwh_sb = p_kwh.tile([C, SL], bf_, name=f"kwh{b}", tag="kwh")
                    nc.scalar.activation(kwh_sb, psk2[0:C, :], AF.Identity,
                                         bias=negku[:], scale=1.0)

                    # S = sum_n v (ones matmul, accumulated over it)
                    po = (b % 2) * 64
                    for it in range(NIT):
                        nc.tensor.matmul(ps[po:po + C, 301:302], v_T[b][it], ones_t,
                                         start=(it == 0), stop=(it == NIT - 1))
                    nc.vector.tensor_scalar_mul(sq[b // 2][po:po + C, :],
                                                ps[po:po + C, 301:302], 0.25)

                    # kq = qT @ k_wh  -> fp8 DR stationary for the qk matmuls
                    ps2 = psH.tile([128, 512], f32, name=f"kq{b}", tag="psH")
                    nc.tensor.matmul(ps2[0:C + 1, 0:SL], sb_qTT, kwh_sb,
                                     start=True, stop=True)
                    nc.scalar.copy(kq_dr[0:C + 1, 0, b * SL:(b + 1) * SL],
                                   ps2[0:C + 1, 0:SL])

                # ---------------- P1: power ladders + transposes ----------------
                nc.gpsimd.memset(v_lad[:, :, :, 0:1], 1.0)
                for it in range(NIT):
                    nc.gpsimd.memset(t_it[it][:, :, 0:1], 1.0)
                for k in range(2, DEG + 1):
                    nc.gpsimd.tensor_tensor(v_lad[:, :, :, k], v_lad[:, :, :, k - 1],
                                            v_lad[:, :, :, 1], op=ALU.mult)
                for it in range(NIT):
                    for k in range(2, DEG + 1):
                        nc.gpsimd.tensor_tensor(t_it[it][:, :, k], t_it[it][:, :, k - 1],
                                                t_it[it][:, :, 1], op=ALU.mult)

                # V transposes: [128 j, (b,k)] -> [(b,k), j] with fp8 hi/lo
                for gen in range(8):
                    psv = psH.tile([128, 512], f32, name=f"vt{gen}", tag="psH")
                    for j4 in range(4):
                        jt = gen * 4 + j4
                        nc.tensor.transpose(psv[0:64, j4 * 128:(j4 + 1) * 128],
                                            v_lad[:, jt], ident)
                    csl = slice(gen * 512, (gen + 1) * 512)
                    nc.vector.tensor_copy(v_dr[0:64, 0, csl], psv[0:64, :])
                    nc.vector.tensor_tensor(v_dr[0:64, 1, csl], psv[0:64, :],
                                            v_dr[0:64, 0, csl], op=ALU.subtract)

                # U transposes + stationaries
                psu = psH.tile([128, 512], f32, name="ut", tag="psH")
                for it in range(NIT):
                    nc.tensor.transpose(psu[0:64, it * 128:(it + 1) * 128],
                                        t_it[it], ident)
                for x in range(B + 1):
                    auc = sb_au[:, x:x + 1]
                    nc.vector.tensor_scalar_mul(u_st[x][0:64, 0, :],
                                                psu[0:64, 0:SL], auc[0:64])
                    nc.vector.scalar_tensor_tensor(u_st[x][0:64, 1, :], psu[0:64, 0:SL],
                                                   auc[0:64], u_st[x][0:64, 0, :],
                                                   op0=ALU.mult, op1=ALU.subtract)

            # ---------------- P2: qk/exp + poly/g + y ----------------
            # (pools opened after P0/P1 scope closed -> reuse v_lad space)
            p_g0 = top.enter_context(tc.tile_pool(name="p_g0", bufs=16))
            p_g1 = top.enter_context(tc.tile_pool(name="p_g1", bufs=16))
            p_rr = top.enter_context(tc.tile_pool(name="p_rr", bufs=2))
            g_tiles = {}
            out_tiles = {}

            def hd_round(jq, rit):
                itp, s = rit // 2, rit % 2
                i_sl = slice(rit * 128, (rit + 1) * 128)
                rr = p_rr.tile([128, 1024], f32, name=f"rr{jq}_{rit}", tag="rr")
                psd = psH.tile([128, 1024], f32, name=f"hdD{jq}_{rit}", tag="psH")
                for hh in range(2):
                    jsl = slice(jq * 1024 + hh * 512, jq * 1024 + (hh + 1) * 512)
                    nc.tensor.matmul(psd[:, hh * 512:(hh + 1) * 512],
                                     u_st[B][:, :, i_sl], v_dr[:, :, jsl],
                                     start=True, stop=True, perf_mode=DR)
                nc.vector.reciprocal_approx_fast(rr, psd)
                for bb in range(B):
                    key = (bb, itp, jq)
                    if key not in g_tiles:
                        pool = p_g0 if bb < 2 else p_g1
                        g_tiles[key] = pool.tile([128, 2, 1024], f8_,
                                                 name=f"g{bb}_{itp}_{jq}", tag="g")
                    psh_ = psH.tile([128, 1024], f32,
                                    name=f"hd{bb}_{jq}_{rit}", tag="psH")
                    for hh in range(2):
                        jsl = slice(jq * 1024 + hh * 512, jq * 1024 + (hh + 1) * 512)
                        nc.tensor.matmul(psh_[:, hh * 512:(hh + 1) * 512],
                                         u_st[bb][:, :, i_sl], v_dr[:, :, jsl],
                                         start=True, stop=True, perf_mode=DR)
                    nc.vector.tensor_tensor(g_tiles[key][:, s, :], psh_, rr,
                                            op=ALU.mult)

            def y_round(pair, j5, tail=False):
                jsl = slice(j5 * 512, (j5 + 1) * 512)
                jq, jh = j5 // 2, j5 % 2
                if tail:
                    ps = psQ.tile([128, 1024], f32, name=f"y{pair}_{j5}", tag="psQ")
                else:
                    ps = psH.tile([128, 512], f32, name=f"y{pair}_{j5}", tag="psH")
                reg = ps[:, 0:512]
                for i in range(2):
                    bb = 2 * pair + i
                    nc.tensor.matmul(reg, sb_wT[:, :, i, :], dr_mov(xf8[bb], jsl),
                                     start=(i == 0), stop=False, perf_mode=DR)
                    for itp in range(NITP):
                        nc.tensor.matmul(reg, vst(v1p_all, bb, itp),
                                         f1[bb][itp][:, :, jsl],
                                         start=False, stop=False, perf_mode=DR)
                        nc.tensor.matmul(
                            reg, vst(vT4_all, bb, itp),
                            g_tiles[(bb, itp, jq)][:, :, jh * 512:(jh + 1) * 512],
                            start=False,
                            stop=(i == 1 and itp == NITP - 1), perf_mode=DR)
                if (pair, jq) not in out_tiles:
                    out_tiles[(pair, jq)] = p_out.tile(
                        [128, 1024], f32, name=f"o{pair}_{jq}", tag="out_sb")
                out_sb = out_tiles[(pair, jq)]
                osl = slice(jh * 512, (jh + 1) * 512)
                nc.scalar.activation(out_sb[:, osl], ps[:, 0:512], AF.Identity,
                                     bias=sq[pair][:], scale=1.0 / 16.0)
                if jh == 1:
                    for i in range(2):
                        bb = 2 * pair + i
                        nc.sync.dma_start(y_part.ap()[bb][:, jq * 1024:(jq + 1) * 1024],
                                          out_sb[i * 64:i * 64 + C, :])

            # hd pacing: front 2/slot then 1/slot -> jq3 done by slot 13
            hd_q = [(jq, rit) for jq in range(NJQ) for rit in range(NIT)]
            HD_PACE = [2, 2, 1, 1, 1, 1, 1, 1, 1, 1, 1, 1, 1, 1, 0, 0]
            y_done = 0
            slot = 0
            for b in range(B):
                for it in range(NIT):
                    itp, s = it // 2, it % 2
                    rounds_done = 16 - len(hd_q)
                    if slot >= 8 and y_done < 2 * (rounds_done // 4) and y_done < NJ5:
                        y_round(0, y_done)
                        y_done += 1
                    for _ in range(HD_PACE[slot]):
                        if hd_q:
                            hd_round(*hd_q.pop(0))
                    zp = [p_zp.tile([128, 1], f32, name=f"zp{_h}", tag="zp")
                          for _h in range(4)]
                    for quar in range(4):
                        psk = psQ.tile([128, 1024], f32, name=f"qk{b}_{it}_{quar}",
                                       tag="psQ")
                        for q2 in range(2):
                            j0 = quar * 1024 + q2 * 512
                            nc.tensor.matmul(
                                psk[:, q2 * 512:(q2 + 1) * 512],
                                kq_dr[:, :, b * SL + it * 128:b * SL + (it + 1) * 128],
                                dr_mov(xf8[b], slice(j0, j0 + 512)),
                                start=True, stop=True, perf_mode=DR)
                        nc.scalar.activation(
                            f1[b][itp][:, s, quar * 1024:(quar + 1) * 1024],
                            psk, AF.Exp, bias=bias_t[:], accum_out=zp[quar])
                    z1a = p_sm.tile([128, 1], f32, name="z1a", tag="z1a")
                    z1b = p_sm.tile([128, 1], f32, name="z1b", tag="z1b")
                    z1 = p_sm.tile([128, 1], f32, name="z1", tag="z1")
                    rz = p_sm.tile([128, 1], f32, name="rz", tag="rz")
                    nc.gpsimd.tensor_tensor(z1a, zp[0], zp[1], op=ALU.add)
                    nc.gpsimd.tensor_tensor(z1b, zp[2], zp[3], op=ALU.add)
                    nc.gpsimd.tensor_tensor(z1, z1a, z1b, op=ALU.add)
                    nc.vector.reciprocal_approx_fast(rz, z1)
                    nc.gpsimd.tensor_scalar(vwr(v1p_all, b, itp, s), v_T[b][it],
                                            scalar1=rz, scalar2=16.0,
                                            op0=ALU.mult, op1=ALU.mult)

                    slot += 1

            while hd_q:
                hd_round(*hd_q.pop(0))
            while y_done < NJ5:
                y_round(0, y_done, tail=True)
                y_done += 1
            for j5 in range(NJ5):
                y_round(1, j5, tail=True)

    nc.compile()
    return nc


@functools.lru_cache(maxsize=1)
def _get_program():
    return _build_program()


def _prep_inputs(inputs):
    x = np.asarray(inputs["x"], np.float32).reshape(B, C, N)
    ones = np.ones((B, 1, N), np.float32)
    x65 = np.concatenate([x, ones], axis=1)                         # [B,65,N]
    x_ext = x65.astype(BF16)
    x_f8 = np.zeros((B, 128, N), F8)
    x_f8[:, :C + 1] = x65.astype(F8)

    qw = np.asarray(inputs["qw"], np.float32)
    qb = np.asarray(inputs["qb"], np.float32)
    kw = np.asarray(inputs["kw"], np.float32)
    kb = np.asarray(inputs["kb"], np.float32)
    mw = np.asarray(inputs["mw"], np.float32)
    mb = np.asarray(inputs["mb"], np.float32)
    vw = np.asarray(inputs["vw"], np.float32)
    vb = np.asarray(inputs["vb"], np.float32)
    ww = np.asarray(inputs["ww"], np.float32)
    wb = np.asarray(inputs["wb"], np.float32)
    g = np.asarray(inputs["bn_gamma"], np.float32)
    be = np.asarray(inputs["bn_beta"], np.float32)
    rm = np.asarray(inputs["bn_rm"], np.float32)
    rv = np.asarray(inputs["bn_rv"], np.float32)

    qTTa = np.concatenate([qw, qb[:, None]], axis=1)                # [64,65]
    kTa = np.concatenate([kw.T, kb[None, :]], axis=0)               # [65,64]

    vmT = np.zeros((C + 1, C + 1), np.float32)
    vmT[:C, :C] = vw.T
    vmT[C, :C] = vb
    vmT[:C, C] = mw[0]
    vmT[C, C] = mb[0]

    mwT = np.concatenate([mw[0][:, None], mb[:, None]], axis=0)     # [65,1]

    inv = g / np.sqrt(rv + EPS)
    wT_bn = np.zeros((C + 1, C), np.float32)
    wT_bn[:C, :] = (ww * inv[:, None]).T
    wT_bn[C, :] = wb * inv + be - rm * inv
    wT16 = np.zeros((128, 2, 2, 2 * C), np.float32)
    for i in range(2):
        wT16[0:C + 1, 0, i, i * C:(i + 1) * C] = (16.0 / N_CORES) * wT_bn

    # au rows ordered (b'*16 + k): weights for the (m/2)^k power basis.
    au = np.zeros((64, 8), np.float32)
    for bp in range(B):
        for k in range(DEG + 1):
            a4 = POLY_A[k] * (4.0 ** k)
            for bt in range(B):
                au[bp * KP + k, bt] = a4 * (0.75 if bp == bt else -0.25)
            au[bp * KP + k, 4] = a4 * 0.25
    zeros8_a = np.zeros((64, 8192), F8)

    common = {
        "x_ext": x_ext,
        "x_f8": x_f8,
        "qTT": qTTa.astype(BF16),
        "kT": kTa.astype(BF16),
        "vmT": vmT.astype(BF16),
        "mwT": mwT.astype(BF16),
        "wT16": wT16.astype(F8),
        "au_in": au,
        "zeros8": zeros8_a,
    }
    in_maps = []
    for ic in range(N_CORES):
        m = dict(common)
        m["xsl_ext"] = np.ascontiguousarray(x_ext[:, :, ic * SL:(ic + 1) * SL])
        in_maps.append(m)
    return in_maps


def kernel(**inputs):
    from concourse.bass_utils import run_bass_kernel_spmd

    nc = _get_program()
    in_maps = _prep_inputs(inputs)
    res = run_bass_kernel_spmd(nc, in_maps, core_ids=list(range(N_CORES)))
    y = np.zeros((B, C, N), np.float32)
    for r in res.results:
        y += r["y_part"]
    return y.reshape(B, C, H, W)


if __name__ == "__main__":
    rng = np.random.default_rng(0)
    ins = {
        "x": rng.standard_normal((B, C, H, W), dtype=np.float32),
        "qw": rng.standard_normal((C, C), dtype=np.float32) * 0.05,
        "qb": rng.standard_normal((C,), dtype=np.float32) * 0.05,
        "kw": rng.standard_normal((C, C), dtype=np.float32) * 0.05,
        "kb": rng.standard_normal((C,), dtype=np.float32) * 0.05,
        "mw": rng.standard_normal((1, C), dtype=np.float32) * 0.05,
        "mb": rng.standard_normal((1,), dtype=np.float32) * 0.05,
        "vw": rng.standard_normal((C, C), dtype=np.float32) * 0.05,
        "vb": rng.standard_normal((C,), dtype=np.float32) * 0.05,
        "ww": rng.standard_normal((C, C), dtype=np.float32) * 0.05,
        "wb": rng.standard_normal((C,), dtype=np.float32) * 0.05,
        "bn_gamma": np.ones((C,), np.float32),
        "bn_beta": np.zeros((C,), np.float32),
        "bn_rm": np.zeros((C,), np.float32),
        "bn_rv": np.ones((C,), np.float32),
    }
    out = kernel(**ins)
    print("kernel output", out.shape, out.dtype, np.abs(out).mean())
